# revision 1
# baseline (speedup 1.0000x reference)
"""Trainium2 Bass kernel for AdaptiveChannelFeatureFusion.

Strategy (8 NeuronCores, channel-sharded -> zero collectives):
  - Each core owns 32 of the 256 channels, all 32 batch images.
  - Depthwise 3x3 convs computed as banded-Toeplitz matmuls on TensorE:
    contraction over H (rows on partitions), the 3 column taps handled by
    free-axis shifts of a 66-wide zero-padded image layout.
  - Product-of-convs reduced via the polarization identity
        conv(x,wk)*conv(x,wq) = ((conv(x,wk+wq))^2 - (conv(x,wk-wq))^2)/4
    so ScalarE Square + a +/-0.25-ones matmul produce lambda directly.
  - GAP via a ones matmul; per-image w-sums finished by one VectorE reduce
    after gathering 1-row PSUM results across partitions with small DMAs.
  - softmax over {a,f} == sigmoid(lam_a - lam_f); BatchNorm stats are local
    (per-channel over batch) so everything stays on-core.
"""

import sys
import numpy as np

sys.path.insert(0, "/opt/trn_rl_repo")

B, C, H, W = 32, 256, 64, 64
NCORES = 8
C_SH = C // NCORES  # 32 channels per core
BN_EPS = 1e-5

_CACHE = {}


def _build_graph(repeat=1, dummy_x=False):
    import concourse.bass as bass
    import concourse.bacc as bacc
    import concourse.tile as tile
    from concourse import mybir

    f32 = mybir.dt.float32
    bf16 = mybir.dt.bfloat16
    AF = mybir.ActivationFunctionType
    OP = mybir.AluOpType
    AX = mybir.AxisListType

    nc = bacc.Bacc("TRN2", target_bir_lowering=False)

    # DRAM parameters (per-core shard shapes)
    # x layout host-prepped: [c, half, par, h, j, 66] with zero pad cols 0,65
    if dummy_x:
        xa_ext = nc.dram_tensor("xa_int", [C_SH, 2, 2, H, 8, 66], f32)
        xf_ext = nc.dram_tensor("xf_int", [C_SH, 2, 2, H, 8, 66], f32)
    else:
        xa_ext = nc.declare_dram_parameter("xa", [C_SH, 2, 2, H, 8, 66], f32, isOutput=False)
        xf_ext = nc.declare_dram_parameter("xf", [C_SH, 2, 2, H, 8, 66], f32, isOutput=False)
    # Toeplitz stationaries: [c, dc, h(64), m(128 = Tp|Tm)]
    st_ext = nc.declare_dram_parameter("stat", [C_SH, 3, H, 128], f32, isOutput=False)
    gamma_ext = nc.declare_dram_parameter("gamma", [C_SH, 1], f32, isOutput=False)
    beta_ext = nc.declare_dram_parameter("beta", [C_SH, 1], f32, isOutput=False)
    out_ext = nc.declare_dram_parameter("out", [C_SH, B], f32, isOutput=True)

    with tile.TileContext(nc) as tc:
        with (
            tc.tile_pool(name="singles", bufs=1) as singles,
            tc.tile_pool(name="xpool", bufs=8) as xpool,
            tc.tile_pool(name="sqpool", bufs=4) as sqpool,
            tc.tile_pool(name="psum", bufs=2, space="PSUM") as psum,
            tc.tile_pool(name="psum2", bufs=4, space="PSUM") as psum2,
            tc.tile_pool(name="res", bufs=1) as res,
        ):
            # resident constants
            stat_sb = singles.tile([128, C_SH, 3, 128], f32)
            # same stationary content on both partition halves (for the two
            # row-tiles of the PE array)
            nc.sync.dma_start(out=stat_sb[0:64, :, :, :], in_=st_ext[:, :, :, :].rearrange("c d h m -> h c d m"))
            nc.sync.dma_start(out=stat_sb[64:128, :, :, :], in_=st_ext[:, :, :, :].rearrange("c d h m -> h c d m"))

            # col 0 = +/-0.25 combine weights; cols 1-31 zero-fill so the lam
            # matmuls initialize full 32-partition PSUM ranges (the copy below
            # reads [0:66] and PSUM must not be read uninitialized)
            ones_pm = singles.tile([128, 32], f32)
            nc.vector.memset(ones_pm[:, :], 0.0)
            nc.vector.memset(ones_pm[0:64, 0:1], 0.25)
            nc.vector.memset(ones_pm[64:128, 0:1], -0.25)

            ones_gap = singles.tile([128, 2], f32)
            nc.vector.memset(ones_gap[:, :], 0.0)
            nc.vector.memset(ones_gap[0:64, 0:1], 1.0 / 4096.0)
            nc.vector.memset(ones_gap[64:128, 1:2], 1.0 / 4096.0)

            gamma_sb = singles.tile([C_SH, 1], f32)
            beta_sb = singles.tile([C_SH, 1], f32)
            eps_sb = singles.tile([C_SH, 1], f32)
            nc.sync.dma_start(out=gamma_sb[:, :], in_=gamma_ext[:, :])
            nc.sync.dma_start(out=beta_sb[:, :], in_=beta_ext[:, :])
            nc.vector.memset(eps_sb[:, :], BN_EPS)

            # per-iter reduced results: [row-partition, iter, j]
            # row-partitions: 0 = lam-even, 32 = lam-odd, 64/65 = gap-e/o
            redc = res.tile([66, 128, 8], f32)

            for rep in range(repeat):
             for c in range(C_SH):
                for t in range(2):
                    x_ext = xa_ext if t == 0 else xf_ext
                    for half in range(2):
                        it = (t * 2 + half) * 32 + c

                        x_tile = xpool.tile([128, 8, 66], f32)
                        dma_eng = nc.sync if (it % 2 == 0) else nc.scalar
                        dma_eng.dma_start(
                            out=x_tile[:, :, :],
                            in_=x_ext[c, half].rearrange("par h j w -> (par h) j w"),
                        )

                        peo = psum.tile([128, 2, 8, 64], f32, tag="peo")
                        for dc in range(3):
                            kw = dict(start=(dc == 0), stop=(dc == 2))
                            nc.tensor.matmul(
                                out=peo[:, 0, :, :],
                                lhsT=stat_sb[0:64, c, dc, :],
                                rhs=x_tile[0:64, :, dc:dc + 64],
                                tile_position=(0, 0),
                                **kw,
                            )
                            nc.tensor.matmul(
                                out=peo[:, 1, :, :],
                                lhsT=stat_sb[64:128, c, dc, :],
                                rhs=x_tile[64:128, :, dc:dc + 64],
                                tile_position=(64, 0),
                                **kw,
                            )

                        sq_eo = sqpool.tile([128, 2, 8, 64], f32, tag="sqeo")
                        sq_flat = sq_eo[:, :, :, :].rearrange("p a j w -> p (a j w)")
                        nc.vector.tensor_copy(out=sq_flat, in_=peo[:, :, :, :].rearrange("p a j w -> p (a j w)"))
                        nc.vector.tensor_tensor(out=sq_flat, in0=sq_flat, in1=sq_flat, op=OP.mult)

                        rows = psum2.tile([128, 512], f32, tag="rows")
                        nc.tensor.matmul(
                            out=rows[0:32, :], lhsT=ones_pm[:, :], rhs=sq_eo[:, 0, :, :],
                            start=True, stop=True, tile_position=(0, 0),
                        )
                        nc.tensor.matmul(
                            out=rows[32:64, :], lhsT=ones_pm[:, :], rhs=sq_eo[:, 1, :, :],
                            start=True, stop=True, tile_position=(0, 32),
                        )
                        nc.tensor.matmul(
                            out=rows[64:66, :], lhsT=ones_gap[:, :], rhs=x_tile[:, :, 1:65],
                            start=True, stop=True, tile_position=(0, 64),
                        )

                        # fused level-2: per-image w-sums straight out of PSUM
                        nc.vector.tensor_reduce(
                            out=redc[:, it, :],
                            in_=rows[0:66, :].rearrange("p (j w) -> p j w", w=64),
                            axis=AX.X,
                            op=OP.add,
                        )

            # rearrange to [c, t, b] with b = 16*half + 2*j + par via
            # partition-scatter SB2SB DMAs (1 src partition -> 32 dst partitions)
            fin_lam = res.tile([C_SH, 2, B], f32)
            fin_gap = res.tile([C_SH, 2, B], f32)
            fl = fin_lam.rearrange("c t (half j par) -> c t half j par", half=2, par=2)
            fg = fin_gap.rearrange("c t (half j par) -> c t half j par", half=2, par=2)
            for t in range(2):
                for half in range(2):
                    p0 = (t * 2 + half) * 32
                    for par in range(2):
                        nc.sync.dma_start(
                            out=fl[:, t, half, :, par],
                            in_=redc[32 * par:32 * par + 1, p0:p0 + 32, :],
                        )
                        nc.sync.dma_start(
                            out=fg[:, t, half, :, par],
                            in_=redc[64 + par:65 + par, p0:p0 + 32, :],
                        )

            # fused = gap_f + sigmoid(lam_a - lam_f) * (gap_a - gap_f)
            d = res.tile([C_SH, B], f32)
            s = res.tile([C_SH, B], f32)
            g = res.tile([C_SH, B], f32)
            fused = res.tile([C_SH, B], f32)
            nc.vector.tensor_tensor(out=d[:, :], in0=fin_lam[:, 0, :], in1=fin_lam[:, 1, :], op=OP.subtract)
            nc.scalar.activation(out=s[:, :], in_=d[:, :], func=AF.Sigmoid)
            nc.vector.tensor_tensor(out=g[:, :], in0=fin_gap[:, 0, :], in1=fin_gap[:, 1, :], op=OP.subtract)
            nc.vector.tensor_tensor(out=g[:, :], in0=s[:, :], in1=g[:, :], op=OP.mult)
            nc.vector.tensor_tensor(out=fused[:, :], in0=g[:, :], in1=fin_gap[:, 1, :], op=OP.add)

            # BatchNorm1d training mode over batch (free axis), per channel
            mu = res.tile([C_SH, 1], f32)
            var = res.tile([C_SH, 1], f32)
            junk = res.tile([C_SH, B], f32)
            cen = res.tile([C_SH, B], f32)
            o_sb = res.tile([C_SH, B], f32)
            nc.vector.tensor_reduce(out=mu[:, :], in_=fused[:, :], axis=AX.X, op=OP.add)
            nc.vector.tensor_scalar(out=mu[:, :], in0=mu[:, :], scalar1=1.0 / B, scalar2=None, op0=OP.mult)
            nc.vector.tensor_scalar(out=cen[:, :], in0=fused[:, :], scalar1=mu[:, :], scalar2=None, op0=OP.subtract)
            nc.vector.tensor_tensor(out=junk[:, :], in0=cen[:, :], in1=cen[:, :], op=OP.mult)
            nc.vector.tensor_reduce(out=var[:, :], in_=junk[:, :], axis=AX.X, op=OP.add)
            nc.scalar.activation(out=var[:, :], in_=var[:, :], func=AF.Sqrt, bias=eps_sb[:, :], scale=1.0 / B)
            nc.vector.reciprocal(out=var[:, :], in_=var[:, :])
            nc.vector.tensor_tensor(out=var[:, :], in0=var[:, :], in1=gamma_sb[:, :], op=OP.mult)
            nc.vector.tensor_scalar(
                out=o_sb[:, :], in0=cen[:, :], scalar1=var[:, :], scalar2=beta_sb[:, :],
                op0=OP.mult, op1=OP.add,
            )
            nc.sync.dma_start(out=out_ext[:, :], in_=o_sb[:, :])

    nc.compile()
    return nc


def _prep_inputs(xa, xf, wk, wq, gamma, beta):
    """Host-side sharding + layout prep. Returns in_maps for 8 cores."""
    wp = (wk + wq)[:, 0].astype(np.float32)  # [C,3,3]
    wm = (wk - wq)[:, 0].astype(np.float32)

    # x: [B, C, H, W] -> per core [c, half, par, h, j, 66] where
    # b = 16*half + 2*j + par
    def prep_x(x, c0):
        xs = x[:, c0:c0 + C_SH]                      # [B, 32, H, W]
        xr = xs.transpose(1, 0, 2, 3)                # [c, b, h, w]
        xr = xr.reshape(C_SH, 2, 8, 2, H, W)         # [c, half, j, par, h, w]
        xr = xr.transpose(0, 1, 3, 4, 2, 5)          # [c, half, par, h, j, w]
        out = np.zeros((C_SH, 2, 2, H, 8, 66), dtype=np.float32)
        out[..., 1:65] = xr
        return np.ascontiguousarray(out)

    in_maps = []
    hh = np.arange(H)
    for core in range(NCORES):
        c0 = core * C_SH
        stat = np.zeros((C_SH, 3, H, 128), dtype=np.float32)
        for dcx in range(3):
            for dh in (-1, 0, 1):
                hp = hh - dh
                v = (hp >= 0) & (hp < H)
                stat[:, dcx, hh[v], hp[v]] = wp[c0:c0 + C_SH, dh + 1, dcx][:, None]
                stat[:, dcx, hh[v], 64 + hp[v]] = wm[c0:c0 + C_SH, dh + 1, dcx][:, None]
        in_maps.append({
            "xa": prep_x(xa, c0),
            "xf": prep_x(xf, c0),
            "stat": stat,
            "gamma": np.ascontiguousarray(gamma[c0:c0 + C_SH].reshape(C_SH, 1).astype(np.float32)),
            "beta": np.ascontiguousarray(beta[c0:c0 + C_SH].reshape(C_SH, 1).astype(np.float32)),
        })
    return in_maps


def run(xa, xf, wk, wq, gamma, beta, trace=False, **trace_kwargs):
    from concourse.bass_utils import run_bass_kernel_spmd

    if "nc" not in _CACHE:
        _CACHE["nc"] = _build_graph()
    nc = _CACHE["nc"]
    in_maps = _prep_inputs(xa, xf, wk, wq, gamma, beta)
    res = run_bass_kernel_spmd(nc, in_maps, core_ids=list(range(NCORES)), trace=trace, **trace_kwargs)
    outs = [res.results[i]["out"] for i in range(NCORES)]  # [c, b] each
    full = np.concatenate([o.T for o in outs], axis=1)      # [B, C]
    return full.astype(np.float32), res


def kernel(xa, xf, wk, wq, gamma, beta):
    out, _ = run(
        np.asarray(xa, dtype=np.float32), np.asarray(xf, dtype=np.float32),
        np.asarray(wk, dtype=np.float32), np.asarray(wq, dtype=np.float32),
        np.asarray(gamma, dtype=np.float32), np.asarray(beta, dtype=np.float32),
    )
    return out



# revision 13
# speedup vs baseline: 3.9398x; 3.9398x over previous
"""Trainium2 Bass kernel for AdaptiveChannelFeatureFusion (v3, fp16).

Strategy (8 NeuronCores, channel-sharded -> zero collectives):
  - Each core owns 32 of the 256 channels, all 32 batch images.
  - Inputs shipped as fp16 (halves HBM traffic); depthwise 3x3 convs as
    banded-Toeplitz matmuls on TensorE (fp16 operands, fp32 PSUM).
  - Polarization identity: conv_k*conv_q = ((conv(k+q))^2 - (conv(k-q))^2)/4.
    Squares on ScalarE (activation Square, PSUM->SBUF); w-reduction of the
    squares on VectorE into per-tile slot columns.
  - GAP rides the TensorE: per-tile selector matmuls (1/4096 weights, one
    column pair per tile) accumulate h-sums of 16 consecutive tiles into one
    [32, 512] PSUM group, so VectorE only reduces once per 16 tiles.
  - Final ones(+-1/4) matmuls contract the (sign, h) partition axis of the
    slot buffer; sigmoid/BN finale identical to the reference math.
"""

import sys
import numpy as np

sys.path.insert(0, "/opt/trn_rl_repo")

B, C, H, W = 32, 256, 64, 64
NCORES = 8
C_SH = C // NCORES  # 32 channels per core
BN_EPS = 1e-5

_CACHE = {}


def _build_graph():
    import concourse.bass as bass
    import concourse.bacc as bacc
    import concourse.tile as tile
    from concourse import mybir

    f32 = mybir.dt.float32
    f16 = mybir.dt.float16
    AF = mybir.ActivationFunctionType
    OP = mybir.AluOpType
    AX = mybir.AxisListType

    nc = bacc.Bacc("TRN2", target_bir_lowering=False)

    xa_ext = nc.declare_dram_parameter("xa", [C_SH, 2, 2, H, 8, 66], f16, isOutput=False)
    xf_ext = nc.declare_dram_parameter("xf", [C_SH, 2, 2, H, 8, 66], f16, isOutput=False)
    # Toeplitz stationaries: [c, dc, h(64), m(128 = Tp|Tm)]
    st_ext = nc.declare_dram_parameter("stat", [C_SH, 3, H, 128], f16, isOutput=False)
    gamma_ext = nc.declare_dram_parameter("gamma", [C_SH, 1], f32, isOutput=False)
    beta_ext = nc.declare_dram_parameter("beta", [C_SH, 1], f32, isOutput=False)
    out_ext = nc.declare_dram_parameter("out", [C_SH, B], f32, isOutput=True)

    NT = 128  # tiles per core: 32 c x 2 jh x 2 t

    with tile.TileContext(nc) as tc:
        with (
            tc.tile_pool(name="singles", bufs=1) as singles,
            tc.tile_pool(name="xpool", bufs=6) as xpool,
            tc.tile_pool(name="sqpool", bufs=4) as sqpool,
            tc.tile_pool(name="psum", bufs=3, space="PSUM") as psum,
            tc.tile_pool(name="gpsum", bufs=1, space="PSUM") as gpsum,
            tc.tile_pool(name="fpsum", bufs=1, space="PSUM") as fpsum,
            tc.tile_pool(name="res", bufs=1) as res,
        ):
            # resident constants; stat chunked so c=0 weights land quickly
            stat_sb = singles.tile([128, C_SH, 3, 128], f16)
            CCH = 8
            for c0 in range(0, C_SH, CCH):
                src = st_ext[c0:c0 + CCH, :, :, :].rearrange("c d h m -> h c d m")
                nc.scalar.dma_start(out=stat_sb[0:64, c0:c0 + CCH, :, :], in_=src)
                nc.scalar.dma_start(out=stat_sb[64:128, c0:c0 + CCH, :, :], in_=src)

            # final-combine weights: +-1/4 for Ep|Em partition halves
            ones_pm = singles.tile([128, 1], f32)
            nc.vector.memset(ones_pm[0:64, :], 0.25)
            nc.vector.memset(ones_pm[64:128, :], -0.25)

            # gap selector stationaries: col 2r+par = 1/4096 on par-half rows
            gsel = singles.tile([128, 16, 32], f16)
            nc.vector.memset(gsel[:, :, :], 0.0)
            for r in range(16):
                nc.vector.memset(gsel[0:64, r, 2 * r:2 * r + 1], 1.0 / 4096.0)
                nc.vector.memset(gsel[64:128, r, 2 * r + 1:2 * r + 2], 1.0 / 4096.0)

            gamma_sb = singles.tile([C_SH, 1], f32)
            beta_sb = singles.tile([C_SH, 1], f32)
            eps_sb = singles.tile([C_SH, 1], f32)
            nc.scalar.dma_start(out=gamma_sb[:, :], in_=gamma_ext[:, :])
            nc.scalar.dma_start(out=beta_sb[:, :], in_=beta_ext[:, :])
            nc.vector.memset(eps_sb[:, :], BN_EPS)

            # slot buffers
            redsq = res.tile([128, NT, 16], f32)   # (sign,h) x (tile) x (par,j)
            gapred = res.tile([32, 8, 8], f32)     # (r,par) x group x j
            # finale operands in native (c, jh, t, par, j) layout
            fin_lam2 = res.tile([C_SH, 2, 2, 2, 8], f32)
            fin_gap2 = res.tile([C_SH, 2, 2, 2, 8], f32)

            gacc_cur = None
            for c in range(C_SH):
                for jh in range(2):
                    xt = []
                    for t in range(2):
                        x_ext = xa_ext if t == 0 else xf_ext
                        x_tile = xpool.tile([128, 8, 66], f16, tag=f"x{t}", name=f"x{t}")
                        nc.sync.dma_start(
                            out=x_tile[:, :, :],
                            in_=x_ext[c, jh].rearrange("par h j w -> (par h) j w"),
                        )
                        xt.append(x_tile)

                    peo = [psum.tile([128, 2, 8, 64], f32, tag="peo", name=f"peo{t}")
                           for t in range(2)]
                    for dc in range(3):
                        kw = dict(start=(dc == 0), stop=(dc == 2))
                        for t in range(2):
                            nc.tensor.matmul(
                                out=peo[t][:, 0, :, :],
                                lhsT=stat_sb[0:64, c, dc, :],
                                rhs=xt[t][0:64, :, dc:dc + 64],
                                tile_position=(0, 0),
                                **kw,
                            )
                            nc.tensor.matmul(
                                out=peo[t][:, 1, :, :],
                                lhsT=stat_sb[64:128, c, dc, :],
                                rhs=xt[t][64:128, :, dc:dc + 64],
                                tile_position=(64, 0),
                                **kw,
                            )

                    for t in range(2):
                        idx = (c * 2 + jh) * 2 + t
                        r, g = idx % 16, idx // 16

                        # gap: h-contraction on PE, accumulated per 16-tile group
                        if r == 0:
                            gacc_cur = gpsum.tile([32, 8, 64], f32, tag="gacc", name="gacc")
                        nc.tensor.matmul(
                            out=gacc_cur[:, :, :],
                            lhsT=gsel[:, r, :],
                            rhs=xt[t][:, :, 1:65],
                            start=(r == 0), stop=(r == 15),
                            tile_position=(0, 0),
                        )
                        if r == 15:
                            nc.vector.tensor_reduce(
                                out=gapred[:, g, :], in_=gacc_cur[:, :, :],
                                axis=AX.X, op=OP.add,
                            )
                            # gather this group's gap rows into fin_gap2[c]
                            for cc in range(4 * g, 4 * g + 4):
                                nc.sync.dma_start(
                                    out=fin_gap2[cc:cc + 1, :, :, :, :],
                                    in_=gapred[8 * (cc % 4):8 * (cc % 4) + 8, g, :],
                                )

                        sq = sqpool.tile([128, 16, 64], f32, tag="sq", name="sq")
                        nc.scalar.activation(
                            out=sq[:, :, :].rearrange("p a w -> p (a w)"),
                            in_=peo[t][:, :, :, :].rearrange("p a j w -> p (a j w)"),
                            func=AF.Square,
                        )
                        nc.vector.tensor_reduce(
                            out=redsq[:, idx, :], in_=sq[:, :, :], axis=AX.X, op=OP.add,
                        )

            # lam = ones_pm.T @ redsq  (contracts (sign,h); applies +-1/4)
            lam_sb = res.tile([1, 4, 512], f32)
            for k in range(4):
                lam_ps = fpsum.tile([1, 512], f32, tag="lam", name="lam")
                nc.tensor.matmul(
                    out=lam_ps[:, :], lhsT=ones_pm[:, :],
                    rhs=redsq[:, 32 * k:32 * (k + 1), :].rearrange("p i j -> p (i j)"),
                    start=True, stop=True,
                )
                nc.vector.tensor_copy(out=lam_sb[:, k, :], in_=lam_ps[:, :])
                # src free order (c' 8, jh, t, par, j) == fin_lam2 native layout
                nc.sync.dma_start(
                    out=fin_lam2[8 * k:8 * (k + 1), :, :, :, :],
                    in_=lam_sb[:, k, :].rearrange("p (c r) -> p c r", c=8),
                )

            # fused = gap_f + sigmoid(lam_a - lam_f) * (gap_a - gap_f)
            # [C_SH, jh, par, j] tensors (flat b-order (jh, par, j))
            d = res.tile([C_SH, 2, 2, 8], f32)
            s = res.tile([C_SH, 2, 2, 8], f32)
            g2 = res.tile([C_SH, 2, 2, 8], f32)
            fused = res.tile([C_SH, 2, 2, 8], f32)
            la = fin_lam2[:, :, 0, :, :]
            lf = fin_lam2[:, :, 1, :, :]
            ga = fin_gap2[:, :, 0, :, :]
            gf = fin_gap2[:, :, 1, :, :]
            A = d[:, :, :, :]
            nc.vector.tensor_tensor(out=A, in0=la, in1=lf, op=OP.subtract)
            nc.scalar.activation(out=s[:, :, :, :], in_=A, func=AF.Sigmoid)
            nc.vector.tensor_tensor(out=g2[:, :, :, :], in0=ga, in1=gf, op=OP.subtract)
            nc.vector.tensor_tensor(out=g2[:, :, :, :], in0=s[:, :, :, :], in1=g2[:, :, :, :], op=OP.mult)
            nc.vector.tensor_tensor(out=fused[:, :, :, :], in0=g2[:, :, :, :], in1=gf, op=OP.add)

            # BatchNorm1d training mode over batch (free axes), per channel
            mu = res.tile([C_SH, 1], f32)
            var = res.tile([C_SH, 1], f32)
            junk = res.tile([C_SH, 2, 2, 8], f32)
            cen = res.tile([C_SH, 2, 2, 8], f32)
            o_sb = res.tile([C_SH, 2, 2, 8], f32)
            fflat = fused[:, :, :, :].rearrange("c a b e -> c (a b e)")
            cflat = cen[:, :, :, :].rearrange("c a b e -> c (a b e)")
            jflat = junk[:, :, :, :].rearrange("c a b e -> c (a b e)")
            oflat = o_sb[:, :, :, :].rearrange("c a b e -> c (a b e)")
            nc.vector.tensor_reduce(out=mu[:, :], in_=fflat, axis=AX.X, op=OP.add)
            nc.vector.tensor_scalar(out=mu[:, :], in0=mu[:, :], scalar1=1.0 / B, scalar2=None, op0=OP.mult)
            nc.vector.tensor_scalar(out=cflat, in0=fflat, scalar1=mu[:, :], scalar2=None, op0=OP.subtract)
            nc.vector.tensor_tensor(out=jflat, in0=cflat, in1=cflat, op=OP.mult)
            nc.vector.tensor_reduce(out=var[:, :], in_=jflat, axis=AX.X, op=OP.add)
            nc.scalar.activation(out=var[:, :], in_=var[:, :], func=AF.Sqrt, bias=eps_sb[:, :], scale=1.0 / B)
            nc.vector.reciprocal(out=var[:, :], in_=var[:, :])
            nc.vector.tensor_tensor(out=var[:, :], in0=var[:, :], in1=gamma_sb[:, :], op=OP.mult)
            nc.vector.tensor_scalar(
                out=oflat, in0=cflat, scalar1=var[:, :], scalar2=beta_sb[:, :],
                op0=OP.mult, op1=OP.add,
            )
            nc.sync.dma_start(out=out_ext[:, :], in_=oflat)

    nc.compile()
    return nc


def _prep_inputs(xa, xf, wk, wq, gamma, beta):
    """Host-side sharding + layout prep. Returns in_maps for 8 cores."""
    wp = (wk + wq)[:, 0].astype(np.float32)  # [C,3,3]
    wm = (wk - wq)[:, 0].astype(np.float32)

    # x: [B, C, H, W] -> per core [c, jh, par, h, j, 66] where b = 16*jh + 2*j + par
    def prep_x(x, c0):
        xs = x[:, c0:c0 + C_SH]                      # [B, 32, H, W]
        xr = xs.transpose(1, 0, 2, 3)                # [c, b, h, w]
        xr = xr.reshape(C_SH, 2, 8, 2, H, W)         # [c, jh, j, par, h, w]
        xr = xr.transpose(0, 1, 3, 4, 2, 5)          # [c, jh, par, h, j, w]
        out = np.zeros((C_SH, 2, 2, H, 8, 66), dtype=np.float16)
        out[..., 1:65] = xr
        return np.ascontiguousarray(out)

    in_maps = []
    hh = np.arange(H)
    for core in range(NCORES):
        c0 = core * C_SH
        stat = np.zeros((C_SH, 3, H, 128), dtype=np.float32)
        for dcx in range(3):
            for dh in (-1, 0, 1):
                hp = hh - dh
                v = (hp >= 0) & (hp < H)
                stat[:, dcx, hh[v], hp[v]] = wp[c0:c0 + C_SH, dh + 1, dcx][:, None]
                stat[:, dcx, hh[v], 64 + hp[v]] = wm[c0:c0 + C_SH, dh + 1, dcx][:, None]
        in_maps.append({
            "xa": prep_x(xa, c0),
            "xf": prep_x(xf, c0),
            "stat": stat.astype(np.float16),
            "gamma": np.ascontiguousarray(gamma[c0:c0 + C_SH].reshape(C_SH, 1).astype(np.float32)),
            "beta": np.ascontiguousarray(beta[c0:c0 + C_SH].reshape(C_SH, 1).astype(np.float32)),
        })
    return in_maps


def run(xa, xf, wk, wq, gamma, beta, trace=False, **trace_kwargs):
    from concourse.bass_utils import run_bass_kernel_spmd

    if "nc" not in _CACHE:
        _CACHE["nc"] = _build_graph()
    nc = _CACHE["nc"]
    in_maps = _prep_inputs(xa, xf, wk, wq, gamma, beta)
    res = run_bass_kernel_spmd(nc, in_maps, core_ids=list(range(NCORES)), trace=trace, **trace_kwargs)
    # per-core out is [c, 32] in (jh, par, j) free order; b = 16*jh + 2*j + par
    outs = []
    for i in range(NCORES):
        o = res.results[i]["out"].reshape(C_SH, 2, 2, 8)
        outs.append(o.transpose(0, 1, 3, 2).reshape(C_SH, B))
    full = np.concatenate([o.T for o in outs], axis=1)      # [B, C]
    return full.astype(np.float32), res


def kernel(xa, xf, wk, wq, gamma, beta):
    out, _ = run(
        np.asarray(xa, dtype=np.float32), np.asarray(xf, dtype=np.float32),
        np.asarray(wk, dtype=np.float32), np.asarray(wq, dtype=np.float32),
        np.asarray(gamma, dtype=np.float32), np.asarray(beta, dtype=np.float32),
    )
    return out
